# revision 6
# baseline (speedup 1.0000x reference)
"""Nearest-neighbor classifier kernel for 8 TRN2 NeuronCores.

Computes: scores = x @ means.T; out = one_hot(argmax(scores, axis=1), 1000).

Strategy (data-parallel, per sharding hint):
  - shard x row-wise across 8 cores (2048 samples each), replicate means
  - host-side staging: transpose shards so the contraction dim (d=2048) lands
    on SBUF partitions, and pre-round operands to the FP22 grid (round to
    nearest, 11 explicit mantissa bits) so the TensorEngine's fp32r input
    truncation is exact (fp32r streams at full PE rate for N>=256)
  - per core: 16 sample-tiles of 128; scores accumulate over 16 k-chunks into
    two PSUM banks of 500 classes
  - epilogue per bank: DVE max8 + max_index -> top-1 (value, index); banks
    merged on-device (is_ge + select) into one f32 class index per sample;
    host expands indices to the one-hot output (exact 0.0/1.0)
  - PE p-state warmup: dummy matmuls run during the DMA fill so real matmuls
    start at full clock (TRN2 ramps 0.65 -> 1.2 -> 2.4 GHz over ~3us)
  - fill phase is k-outer (matmuls chase the DMA stream chunk by chunk);
    steady state is bank-sequential per tile with per-bank epilogues so PSUM
    frees early and the next group never stalls
"""

import sys

if "/opt/trn_rl_repo" not in sys.path:
    sys.path.insert(0, "/opt/trn_rl_repo")

import numpy as np

import concourse.bass as bass
import concourse.mybir as mybir
from concourse import bacc
from concourse.tile import TileContext
from concourse.bass_utils import run_bass_kernel_spmd

N_CORES = 8
NS_TOTAL = 16384
ND = 2048
NCLS = 1000

P = 128            # SBUF partitions / PE contraction tile
KC = ND // P       # 16 k-chunks
PAIR = 512         # samples per x DMA slab
NP = 2048 // PAIR  # 4 slabs per core (ns=2048)
TPP = PAIR // P    # 4 sample-tiles per slab
CLS = ((0, 500), (500, 1000))  # means col ranges; PSUM offset si*512
NWARM = 14         # p-state warmup matmuls (fp32, 128 cols each)


def _rtn22(a: np.ndarray) -> np.ndarray:
    """Round fp32 to nearest point on the FP22 (11 explicit mantissa bit)
    grid, so the PE's fp32r truncation of the result is the identity."""
    u = a.view(np.uint32)
    u = (u + np.uint32(0x800)) & np.uint32(0xFFFFF000)
    return u.view(np.float32)


def build_bass(ns: int):
    fr = mybir.dt.float32r
    f32 = mybir.dt.float32
    u32 = mybir.dt.uint32

    nc = bacc.Bacc("TRN2", target_bir_lowering=False, debug=False)
    xt = nc.dram_tensor("xt", [ND, ns], fr, kind="ExternalInput")
    mt = nc.dram_tensor("mt", [ND, NCLS], fr, kind="ExternalInput")
    oidx = nc.dram_tensor("oidx", [ns, 1], f32, kind="ExternalOutput")

    with TileContext(nc) as tc:
        with (
            tc.tile_pool(name="means", bufs=1) as mpool,
            tc.tile_pool(name="xin", bufs=2) as xpool,
            tc.tile_pool(name="warm", bufs=1) as wpool,
            tc.tile_pool(name="stats", bufs=4) as spool,
            tc.tile_pool(name="scores", bufs=4, space="PSUM") as pspool,
        ):
            # ---- PE p-state warmup: dummy matmuls run while the first DMA
            # chunks land, so real matmuls start at 2.4GHz. fp32 matmuls
            # stream at 1/4 rate, so a few cover the whole fill window ----
            wt = wpool.tile([P, P], f32, name="wt", tag="wt")
            nc.vector.memset(wt, 0.0)
            psw = pspool.tile([P, 1024], f32, name="psw", tag="ps")
            for _ in range(NWARM):
                nc.tensor.matmul(
                    psw[:, 0:P], wt, wt, start=True, stop=True,
                )

            m_tiles = {}
            x_tiles = {}

            def load_m(k):
                t = mpool.tile([P, NCLS], fr, name=f"m{k}", tag=f"m{k}")
                if k == 0:
                    pieces = ((0, 250), (250, 500), (500, 750), (750, 1000))
                elif k == 1:
                    pieces = CLS
                else:
                    pieces = ((0, NCLS),)
                for lo, hi in pieces:
                    nc.sync.dma_start(
                        out=t[:, lo:hi], in_=mt[k * P:(k + 1) * P, lo:hi]
                    )
                m_tiles[k] = t

            def load_x(p, k, eng):
                t = xpool.tile([P, PAIR], fr, name=f"x{p}_{k}", tag=f"x{k}")
                if p == 0 and k < 2:
                    halves = ((0, 256), (256, 512))
                else:
                    halves = ((0, PAIR),)
                for lo, hi in halves:
                    eng.dma_start(
                        out=t[:, lo:hi],
                        in_=xt[k * P:(k + 1) * P,
                               p * PAIR + lo:p * PAIR + hi],
                    )
                x_tiles[(p, k)] = t

            def mm(ps, p, k, ti, si, start, stop):
                lo, hi = CLS[si]
                nc.tensor.matmul(
                    ps[:, si * 512:si * 512 + (hi - lo)],
                    x_tiles[(p, k)][:, ti * P:(ti + 1) * P],
                    m_tiles[k][:, lo:hi],
                    start=start, stop=stop,
                )

            def epi_bank(ps, si):
                lo, hi = CLS[si]
                w = hi - lo
                mx8 = spool.tile([P, 8], f32, name=f"mx{si}", tag=f"mx{si}")
                nc.vector.max(mx8, ps[:, si * 512:si * 512 + w])
                ix8 = spool.tile([P, 8], u32, name=f"ix{si}", tag=f"ix{si}")
                nc.vector.max_index(ix8, mx8, ps[:, si * 512:si * 512 + w])
                return mx8, ix8

            def epi_merge(row0, mxa, ixa, mxb, ixb):
                # winner = a if mxa >= b else b; bank1 classes start at 500.
                # >= keeps the lower class index on cross-bank ties, matching
                # np.argmax.
                msk = spool.tile([P, 1], mybir.dt.uint8, name="msk", tag="msk")
                nc.vector.tensor_tensor(
                    msk, mxa[:, 0:1], mxb[:, 0:1], mybir.AluOpType.is_ge
                )
                ib = spool.tile([P, 1], f32, name="ib", tag="ib")
                nc.vector.tensor_scalar_add(ib, ixb[:, 0:1], 500.0)
                res = spool.tile([P, 1], f32, name="res", tag="res")
                nc.vector.tensor_copy(res, ib)
                ia = spool.tile([P, 1], f32, name="ia", tag="ia")
                nc.vector.tensor_copy(ia, ixa[:, 0:1])
                nc.vector.copy_predicated(res, msk, ia)
                nc.scalar.dma_start(out=oidx[row0:row0 + P, :], in_=res)

            def epilogue(ps, gtile):
                mxa, ixa = epi_bank(ps, 0)
                mxb, ixb = epi_bank(ps, 1)
                epi_merge(gtile * P, mxa, ixa, mxb, ixb)

            # ---- fill: slab 0 (tiles 0..3), k-outer so matmuls chase the
            # DMA stream; both banks accumulate per chunk ----
            ps_fill = [
                pspool.tile([P, 1024], f32, name=f"ps{t}", tag="ps")
                for t in range(TPP)
            ]
            for k in range(KC - 2):
                load_m(k)
                load_x(0, k, nc.sync)
                for ti in range(TPP):
                    for si in range(2):
                        mm(ps_fill[ti], 0, k, ti, si, k == 0, False)
            for k in (KC - 2, KC - 1):
                load_m(k)
                load_x(0, k, nc.sync)
            # slab 1 prefetch queued behind all fill DMAs (FIFO priority)
            for k in range(KC):
                load_x(1, k, nc.sync)
            # tail tile-major: tile 0 finishes (and frees its PSUM) first
            for ti in range(TPP):
                for k in (KC - 2, KC - 1):
                    for si in range(2):
                        mm(ps_fill[ti], 0, k, ti, si, False, k == KC - 1)
                epilogue(ps_fill[ti], ti)

            # ---- steady state: slabs 1..3, bank-sequential per tile ----
            for p in range(1, NP):
                for ti in range(TPP):
                    ps = pspool.tile(
                        [P, 1024], f32, name=f"ps{p}_{ti}", tag="ps"
                    )
                    for si in range(2):
                        for k in range(KC):
                            mm(ps, p, k, ti, si, k == 0, k == KC - 1)
                    epilogue(ps, p * TPP + ti)
                    if ti == 0 and p + 1 < NP:
                        # next slab's x loads; WAR waits on slot reuse pace
                        # them behind the previous slab's consumption
                        for k in range(KC):
                            load_x(p + 1, k, nc.scalar)

    nc.compile()
    return nc


def run(x, means, trace=False, **spmd_kwargs):
    x = np.ascontiguousarray(np.asarray(x, dtype=np.float32))
    means = np.ascontiguousarray(np.asarray(means, dtype=np.float32))
    assert x.shape == (NS_TOTAL, ND) and means.shape == (NCLS, ND)

    xr = _rtn22(x)
    mr_t = np.ascontiguousarray(_rtn22(means).T)

    ns = NS_TOTAL // N_CORES
    in_maps = []
    for c in range(N_CORES):
        in_maps.append({
            "xt": np.ascontiguousarray(xr[c * ns:(c + 1) * ns, :].T),
            "mt": mr_t,
        })

    nc = build_bass(ns)
    res = run_bass_kernel_spmd(
        nc, in_maps, core_ids=list(range(N_CORES)), trace=trace, **spmd_kwargs
    )
    idx = np.concatenate(
        [r["oidx"][:, 0] for r in res.results], axis=0
    ).astype(np.int64)
    full = np.zeros((NS_TOTAL, NCLS), dtype=np.float32)
    full[np.arange(NS_TOTAL), idx] = 1.0
    return full, res


def kernel(x=None, means=None, n_classes=None, **_ignored) -> np.ndarray:
    assert n_classes is None or int(n_classes) == NCLS
    out, _ = run(x, means)
    return out


# revision 8
# speedup vs baseline: 1.1583x; 1.1583x over previous
"""Nearest-neighbor classifier kernel for 8 TRN2 NeuronCores.

Computes: scores = x @ means.T; out = one_hot(argmax(scores, axis=1), 1000).

Strategy (data-parallel, per sharding hint):
  - shard x row-wise across 8 cores (2048 samples each), replicate means
  - host-side staging: transpose shards so the contraction dim (d=2048) lands
    on SBUF partitions, and pre-round operands to the FP22 grid (round to
    nearest, 11 explicit mantissa bits) so the TensorEngine's fp32r input
    truncation is exact (fp32r streams at full PE rate for N>=256)
  - per core: 16 sample-tiles of 128; scores accumulate over 16 k-chunks into
    two PSUM banks of 500 classes; all four slabs run k-outer so matmuls
    chase the DMA stream and x chunks are consumed progressively
  - every DMA load is triggered from the Sync engine in consumption order so
    HWDGE ring backpressure acts as a priority queue (a later slab can never
    starve an earlier one)
  - epilogue: ScalarE drains PSUM to SBUF (frees the PSUM tile in ~0.9us so
    the next slab never stalls), then DVE max8 + max_index per bank and an
    on-device bank merge produce one f32 class index per sample; a single
    batched DMA ships the [128, 16] index block; host expands one-hot
  - PE p-state warmup: fp32 dummy matmuls run during the DMA fill so real
    matmuls start at 2.4GHz (TRN2 ramps 0.65 -> 1.2 -> 2.4 GHz over ~3us)
"""

import sys

if "/opt/trn_rl_repo" not in sys.path:
    sys.path.insert(0, "/opt/trn_rl_repo")

import numpy as np

import concourse.bass as bass
import concourse.mybir as mybir
from concourse import bacc
from concourse.tile import TileContext
from concourse.bass_utils import run_bass_kernel_spmd

N_CORES = 8
NS_TOTAL = 16384
ND = 2048
NCLS = 1000

P = 128            # SBUF partitions / PE contraction tile
KC = ND // P       # 16 k-chunks
PAIR = 512         # samples per x DMA slab
NP = 2048 // PAIR  # 4 slabs per core (ns=2048)
TPP = PAIR // P    # 4 sample-tiles per slab
NT = NP * TPP      # 16 tiles per core
CLS = ((0, 500), (500, 1000))  # means col ranges; PSUM offset si*512
NWARM = 10         # p-state warmup matmuls (fp32, 128 cols each)


def _rtn22(a: np.ndarray) -> np.ndarray:
    """Round fp32 to nearest point on the FP22 (11 explicit mantissa bit)
    grid, so the PE's fp32r truncation of the result is the identity."""
    u = a.view(np.uint32)
    u = (u + np.uint32(0x800)) & np.uint32(0xFFFFF000)
    return u.view(np.float32)


def build_bass(ns: int):
    fr = mybir.dt.float32r
    f32 = mybir.dt.float32
    u32 = mybir.dt.uint32

    nc = bacc.Bacc("TRN2", target_bir_lowering=False, debug=False)
    xt = nc.dram_tensor("xt", [ND, ns], fr, kind="ExternalInput")
    mt = nc.dram_tensor("mt", [ND, NCLS], fr, kind="ExternalInput")
    oidx = nc.dram_tensor("oidx", [P, NT], f32, kind="ExternalOutput")

    with TileContext(nc) as tc:
        with (
            tc.tile_pool(name="means", bufs=1) as mpool,
            tc.tile_pool(name="xin", bufs=2) as xpool,
            tc.tile_pool(name="warm", bufs=1) as wpool,
            tc.tile_pool(name="drain", bufs=4) as dpool,
            tc.tile_pool(name="stats", bufs=4) as spool,
            tc.tile_pool(name="out", bufs=1) as opool,
            tc.tile_pool(name="scores", bufs=4, space="PSUM") as pspool,
        ):
            # ---- PE p-state warmup: dummy matmuls run while the first DMA
            # chunks land, so real matmuls start at 2.4GHz. fp32 matmuls
            # stream at 1/4 rate, so a few cover the whole fill window ----
            wt = wpool.tile([P, P], f32, name="wt", tag="wt")
            nc.vector.memset(wt, 0.0)
            psw = pspool.tile([P, 1024], f32, name="psw", tag="ps")
            for _ in range(NWARM):
                nc.tensor.matmul(
                    psw[:, 0:P], wt, wt, start=True, stop=True,
                )

            res_all = opool.tile([P, NT], f32, name="res", tag="res")
            m_tiles = {}
            x_tiles = {}

            def load_m(k):
                t = mpool.tile([P, NCLS], fr, name=f"m{k}", tag=f"m{k}")
                if k == 0:
                    pieces = ((0, 250), (250, 500), (500, 750), (750, 1000))
                elif k == 1:
                    pieces = CLS
                else:
                    pieces = ((0, NCLS),)
                for lo, hi in pieces:
                    nc.sync.dma_start(
                        out=t[:, lo:hi], in_=mt[k * P:(k + 1) * P, lo:hi]
                    )
                m_tiles[k] = t

            def load_x(p, k):
                t = xpool.tile([P, PAIR], fr, name=f"x{p}_{k}", tag=f"x{k}")
                if p == 0 and k < 2:
                    halves = ((0, 256), (256, 512))
                else:
                    halves = ((0, PAIR),)
                for lo, hi in halves:
                    nc.sync.dma_start(
                        out=t[:, lo:hi],
                        in_=xt[k * P:(k + 1) * P,
                               p * PAIR + lo:p * PAIR + hi],
                    )
                x_tiles[(p, k)] = t

            def mm(ps, p, k, ti, si, start, stop):
                lo, hi = CLS[si]
                nc.tensor.matmul(
                    ps[:, si * 512:si * 512 + (hi - lo)],
                    x_tiles[(p, k)][:, ti * P:(ti + 1) * P],
                    m_tiles[k][:, lo:hi],
                    start=start, stop=stop,
                )

            def epi_from(src, off, gtile):
                """max8 + max_index per bank from `src` (sbuf copy or psum),
                merge banks, store the tile's index column."""
                banks = []
                for si, (lo, hi) in enumerate(CLS):
                    w = hi - lo
                    s = src[:, off[si]:off[si] + w]
                    mx8 = spool.tile([P, 8], f32, name=f"mx{si}",
                                     tag=f"mx{si}")
                    nc.vector.max(mx8, s)
                    ix8 = spool.tile([P, 8], u32, name=f"ix{si}",
                                     tag=f"ix{si}")
                    nc.vector.max_index(ix8, mx8, s)
                    banks.append((mx8, ix8))
                (mxa, ixa), (mxb, ixb) = banks
                # winner = a if mxa >= mxb else b; bank1 classes start at 500.
                # >= keeps the lower class index on cross-bank ties, matching
                # np.argmax.
                msk = spool.tile([P, 1], mybir.dt.uint8, name="msk", tag="msk")
                nc.vector.tensor_tensor(
                    msk, mxa[:, 0:1], mxb[:, 0:1], mybir.AluOpType.is_ge
                )
                col = res_all[:, gtile:gtile + 1]
                nc.vector.tensor_scalar_add(col, ixb[:, 0:1], 500.0)
                ia = spool.tile([P, 1], f32, name="ia", tag="ia")
                nc.vector.tensor_copy(ia, ixa[:, 0:1])
                nc.vector.copy_predicated(col, msk, ia)

            def epilogue(ps, gtile, last):
                if last:
                    # no one needs this PSUM afterwards: skip the drain copy
                    epi_from(ps, (0, 512), gtile)
                else:
                    # ScalarE drains PSUM fast so the tile frees early
                    sc = dpool.tile([P, 1024], f32, name="sc", tag="sc")
                    nc.scalar.copy(sc[:, 0:500], ps[:, 0:500])
                    nc.scalar.copy(sc[:, 512:1012], ps[:, 512:1012])
                    epi_from(sc, (0, 512), gtile)

            # ---- main loop: all slabs k-outer; chunk loads in consumption
            # order on the sync queue ----
            for p in range(NP):
                ps_t = [
                    pspool.tile([P, 1024], f32, name=f"ps{p}_{t}", tag="ps")
                    for t in range(TPP)
                ]
                for k in range(KC):
                    if p == 0:
                        load_m(k)
                    load_x(p, k)
                    if k < KC - 2:
                        for ti in range(TPP):
                            for si in (0, 1):
                                mm(ps_t[ti], p, k, ti, si, k == 0, False)
                # tail tile-major: each tile finishes (and frees its PSUM)
                # as early as possible
                for ti in range(TPP):
                    for k in (KC - 2, KC - 1):
                        for si in (0, 1):
                            mm(ps_t[ti], p, k, ti, si, False, k == KC - 1)
                    gtile = p * TPP + ti
                    epilogue(ps_t[ti], gtile, gtile == NT - 1)

            nc.scalar.dma_start(out=oidx[:, :], in_=res_all)

    nc.compile()
    return nc


def run(x, means, trace=False, **spmd_kwargs):
    x = np.ascontiguousarray(np.asarray(x, dtype=np.float32))
    means = np.ascontiguousarray(np.asarray(means, dtype=np.float32))
    assert x.shape == (NS_TOTAL, ND) and means.shape == (NCLS, ND)

    xr = _rtn22(x)
    mr_t = np.ascontiguousarray(_rtn22(means).T)

    ns = NS_TOTAL // N_CORES
    in_maps = []
    for c in range(N_CORES):
        in_maps.append({
            "xt": np.ascontiguousarray(xr[c * ns:(c + 1) * ns, :].T),
            "mt": mr_t,
        })

    nc = build_bass(ns)
    res = run_bass_kernel_spmd(
        nc, in_maps, core_ids=list(range(N_CORES)), trace=trace, **spmd_kwargs
    )
    # oidx is [128, 16] per core: column t holds tile t's sample indices
    idx = np.concatenate(
        [r["oidx"].T.reshape(-1) for r in res.results], axis=0
    ).astype(np.int64)
    full = np.zeros((NS_TOTAL, NCLS), dtype=np.float32)
    full[np.arange(NS_TOTAL), idx] = 1.0
    return full, res


def kernel(x=None, means=None, n_classes=None, **_ignored) -> np.ndarray:
    assert n_classes is None or int(n_classes) == NCLS
    out, _ = run(x, means)
    return out
